# revision 41
# baseline (speedup 1.0000x reference)
"""GQA attention block (B=2, S=2048, D=2048, H=32, KVH=8, HD=64, RoPE) on 8
Trainium2 NeuronCores.

Sharding: core = (batch, kv-head pair). Core c handles batch c//4 and kv heads
{2*(c%4), 2*(c%4)+1} (i.e. q heads 8*(c%4)..8*(c%4)+7). Each core runs the full
chain for its heads: q/k/v projections + RoPE, attention, and its row-slice of
the output projection; the host sums the 4 partial wo-outputs per batch.

Device-side layout choices (all host-side transforms are free):
- x is passed transposed (xT [D, S]) so projections produce qT/kT/vT with the
  head dim on partitions.
- RoPE uses the "half layout": wq/wk rows are permuted per head to
  [even dims, odd dims] so the rotation pairs live in 32-partition blocks;
  cos/sin tables are precomputed host-side ([128, S] tiles matching the qT/kT
  partition layout). The 1/sqrt(HD) score scale is folded into the Q tables.
- Scores are computed transposed ([keys, queries] on [partitions, free]) so
  softmax exp is a pure elementwise ACT op (no reduction, no transpose of
  probabilities) and the PV matmul consumes probsT directly as the moving
  operand. Softmax skips max-subtraction (scores are bounded ~|7| at this
  problem's scale; fp32 exp is safe).
- The softmax normalizer z is produced by a ones*mask column appended to V
  (M=65 PV matmul); attention output is divided by z after PV (64x less work
  than normalizing probabilities).
- All matmuls run as float32r (full-rate fp32 streaming mode).
"""

import sys

import numpy as np

if "/opt/trn_rl_repo" not in sys.path:
    sys.path.insert(0, "/opt/trn_rl_repo")

B, S, D = 2, 2048, 2048
H, KVH = 32, 8
HD = D // H            # 64
NREP = H // KVH        # 4
ROPE_THETA = 10000.0
N_CORES = 8
P = 128
NQ = 512               # q rows per core (8 heads * 64)
NKV = 128              # k/v rows per core (2 kv heads * 64)
KO = D // P            # 16 contraction chunks for projections
SC = S // 512          # 4 column chunks of 512
HALF = S // 2          # S-half processed per projection pass


def _rope_tables():
    """cos/sin tables [P, S] matching the qT/kT partition layout.

    Partition layout per 64-row head block: rows 0:32 = "a" (even dims),
    rows 32:64 = "b" (odd dims). a' = a*cos - b*sin ; b' = a*sin + b*cos.
    The in0 of the fused swap-multiply reads the OTHER block, so the sin
    table carries -sin on a-rows and +sin on b-rows.

    Computed in float32 to match the reference's jnp float32 math.
    """
    half = HD // 2
    freqs = (1.0 / (ROPE_THETA **
                    (np.arange(0, HD, 2, dtype=np.float32) / np.float32(HD))))
    freqs = freqs.astype(np.float32)                                  # [32]
    ang = (np.arange(S, dtype=np.float32)[None, :] * freqs[:, None])  # [32, S]
    cos = np.cos(ang).astype(np.float32)
    sin = np.sin(ang).astype(np.float32)
    ctab = np.concatenate([cos, cos, cos, cos], axis=0)               # [128, S]
    stab = np.concatenate([-sin, sin, -sin, sin], axis=0)             # [128, S]
    return ctab, stab


def _build_bass(reps: int = 1):
    import concourse.bass as bass  # noqa: F401
    import concourse.tile as tile
    from concourse import bacc, mybir
    from concourse.masks import make_identity

    F32 = mybir.dt.float32
    F32R = mybir.dt.float32r
    EXP = mybir.ActivationFunctionType.Exp
    COPY = mybir.ActivationFunctionType.Copy
    MULT = mybir.AluOpType.mult
    ADD = mybir.AluOpType.add

    nc = bacc.Bacc("TRN2", target_bir_lowering=False, debug=False,
                   num_devices=N_CORES)

    xT = nc.dram_tensor("xT", [D, S], F32R, kind="ExternalInput")
    wqT = nc.dram_tensor("wqT", [D, NQ], F32R, kind="ExternalInput")
    wkT = nc.dram_tensor("wkT", [D, NKV], F32R, kind="ExternalInput")
    wvT = nc.dram_tensor("wvT", [D, NKV], F32R, kind="ExternalInput")
    woT = nc.dram_tensor("woT", [NQ, D], F32R, kind="ExternalInput")
    ck = nc.dram_tensor("ck", [P, S], F32, kind="ExternalInput")
    sk = nc.dram_tensor("sk", [P, S], F32, kind="ExternalInput")
    maskT = nc.dram_tensor("maskT", [P, KO], F32, kind="ExternalInput")
    part = nc.dram_tensor("part", [S, D], F32, kind="ExternalOutput")

    xT_r = xT.ap().rearrange("(ko p) s -> p ko s", p=P)     # [128, 16, 2048]
    wqT_r = wqT.ap().rearrange("(ko p) m -> p ko m", p=P)   # [128, 16, 512]
    wkT_r = wkT.ap().rearrange("(ko p) m -> p ko m", p=P)   # [128, 16, 128]
    wvT_r = wvT.ap().rearrange("(ko p) m -> p ko m", p=P)   # [128, 16, 128]
    woT_r = woT.ap().rearrange("(dk p) e -> p dk e", p=P)   # [128, 4, 2048]

    with tile.TileContext(nc) as tc:
      for rep in range(reps):
        # pools are scoped so per-partition SBUF stays under 192KB:
        # pass1-only data (wk/wv/k-tables/vsb) frees before attention(0);
        # q-proj data (wq/x/q-tables) frees before attention(1); attn/woT
        # allocate late.
        with tc.tile_pool(name="persist", bufs=1) as persist, \
             tc.tile_pool(name="probs", bufs=4) as prpool, \
             tc.tile_pool(name="nrm", bufs=3) as nrmpool, \
             tc.tile_pool(name="mmPS", bufs=2, space="PSUM") as mmps, \
             tc.tile_pool(name="attnPS", bufs=2, space="PSUM") as spool, \
             tc.tile_pool(name="pvPS", bufs=2, space="PSUM") as pvpool:

            from contextlib import ExitStack as _ES
            qsb = [persist.tile([P, S], F32R, tag=f"qsb{m}", name=f"qsb{m}_{rep}")
                   for m in range(4)]
            kab = persist.tile([P, S], F32R, tag="kab")
            vpr = [persist.tile([P, KO, HD + 1], F32R, tag=f"vpr{i}",
                                name=f"vpr{i}_{rep}")
                   for i in range(2)]
            msk = persist.tile([P, KO], F32, tag="msk")
            ident = persist.tile([P, P], F32, tag="ident")

            nc.sync.dma_start(msk[:], maskT.ap())
            make_identity(nc, ident[:])

            def rope_evac(ps, dst_tile, s0, ctab, stab, swpool, scale):
                """dst[:, s0:s0+512] = rope(ps * scale), tables at cols s0.

                The 1/sqrt(HD) query scale folds in as the scalar of
                scalar_tensor_tensor: out = (in0 * scale) * table."""
                dst = dst_tile[:, s0:s0 + 512]
                c_sl = ctab[:, s0:s0 + 512]
                s_sl = stab[:, s0:s0 + 512]
                sw = swpool.tile([P, 512], F32, tag="sw")
                # fused swap+scale+mult: sw[blk] = ps[other]*scale*stab[blk]
                for o in range(0, P, 64):
                    nc.vector.scalar_tensor_tensor(
                        sw[o:o + 32, :], ps[o + 32:o + 64, :], scale,
                        s_sl[o:o + 32, :], MULT, MULT)
                    nc.vector.scalar_tensor_tensor(
                        sw[o + 32:o + 64, :], ps[o:o + 32, :], scale,
                        s_sl[o + 32:o + 64, :], MULT, MULT)
                nc.vector.scalar_tensor_tensor(
                    dst, ps[:], scale, c_sl, MULT, MULT)
                nc.vector.tensor_tensor(dst, dst, sw[:], ADD)

            KH = KO // 8  # x streams in [P, 2, 512] k-eighth tiles
            NKQ = KO // KH

            def make_xq(xpool, s0):
                out = []
                for kq in range(NKQ):
                    xq = xpool.tile([P, KH, 512], F32R, tag="xq",
                                    name=f"xq{kq}")
                    nc.sync.dma_start(
                        xq[:], xT_r[:, kq * KH:(kq + 1) * KH, s0:s0 + 512])
                    out.append(xq)
                return out

            def proj_mm(ps, xqs, w_sb, mlo, mhi, rot=0):
                # rotate the contraction order so the x quarter-tiles free
                # staggered (enables prefetch of the next column chunk)
                ks = [(rot * KH + i) % KO for i in range(KO)]
                for i, k in enumerate(ks):
                    nc.tensor.matmul(
                        ps[:], w_sb[:, k, mlo:mhi],
                        xqs[k // KH][:, k % KH, :],
                        start=(i == 0), stop=(i == KO - 1))

            def wo_chunk(wot_sb, attn, oevpool, qt):
                for n in range(SC):
                    po = mmps.tile([P, 512], F32, tag="mm")
                    for dk in range(4):
                        nc.tensor.matmul(
                            po[:],
                            attn[dk][:, qt * P:(qt + 1) * P],
                            wot_sb[:, dk, n * 512:(n + 1) * 512],
                            start=(dk == 0), stop=(dk == 3))
                    ot = oevpool.tile([P, 512], F32, tag="ot")
                    nc.vector.tensor_copy(ot[:], po[:])
                    nc.sync.dma_start(
                        part.ap()[qt * P:(qt + 1) * P,
                                  n * 512:(n + 1) * 512],
                        ot[:])

            def attention(qcp, attn, fillers):
                """Attention for query cols qcp*1024:(qcp+1)*1024; calls one
                filler (wo chunk / deferred q-proj) between head iterations
                so that work rides in the ACT-bound phase's PE slack.

                attn ALIASES qsb: head h's normalized output overwrites the
                very q columns it just consumed (rows qb:qb+64 of tile m);
                later heads touch different rows/columns, so the Tile
                dependency tracker keeps this safe."""
                wo_iter = list(fillers)
                for h in range(8):
                    # q heads are packed host-side as [0,4,1,5,2,6,3,7] so
                    # head h sits in qsb tile h%4 at partition base
                    # (h//4)*64, matching its kv head's base in kab (matmul
                    # requires equal lhsT/rhs partition bases).
                    m = h % 4
                    qb = (h // 4) * HD
                    kb = qb
                    vp = vpr[h // 4]
                    pv0 = pvpool.tile([P, 512], F32, tag="pv",
                                      name=f"pv0_{rep}")
                    pv1 = pvpool.tile([P, 512], F32, tag="pv",
                                      name=f"pv1_{rep}")
                    pvs = (pv0, pv1)
                    for kc in range(KO):
                        ss = spool.tile([P, 1024], F32, tag="ss")
                        for j in range(2):
                            qc = qcp * 2 + j
                            nc.tensor.matmul(
                                ss[:, j * 512:(j + 1) * 512],
                                kab[kb:kb + HD, kc * P:(kc + 1) * P],
                                qsb[m][qb:qb + HD,
                                       qc * 512:(qc + 1) * 512],
                                start=True, stop=True)
                        pr = prpool.tile([P, 1024], F32R, tag="pr")
                        nc.scalar.activation(pr[:], ss[:], EXP)
                        for j in range(2):
                            nc.tensor.matmul(
                                pvs[j][0:HD + 1, :],
                                vp[:, kc, :],
                                pr[:, j * 512:(j + 1) * 512],
                                start=(kc == 0), stop=(kc == KO - 1))
                    for j in range(2):
                        qc = qcp * 2 + j
                        # stage the whole PV result to SBUF in one copy so
                        # the psum bank frees for the next head immediately
                        # (also: custom-DVE ops misread partition-base != 0
                        # inputs on HW, so the z row must reach base-0 SBUF
                        # before reciprocal).
                        pvs_sb = nrmpool.tile([HD + 1, 512], F32, tag="pvsb")
                        nc.vector.tensor_copy(pvs_sb[:], pvs[j][0:HD + 1, :])
                        zrow = nrmpool.tile([1, 512], F32, tag="zrow")
                        nc.vector.tensor_copy(zrow[:], pvs_sb[HD:HD + 1, :])
                        rz = nrmpool.tile([1, 512], F32, tag="rz")
                        nc.vector.reciprocal_approx_fast(rz[:], zrow[:])
                        rzb = nrmpool.tile([HD, 512], F32, tag="rzb")
                        nc.gpsimd.partition_broadcast(rzb[:], rz[:])
                        nc.vector.tensor_tensor(
                            attn[m][qb:qb + HD, qc * 512:(qc + 1) * 512],
                            pvs_sb[0:HD, :], rzb[:], MULT)
                    if wo_iter:
                        wo_iter.pop(0)()
                for f in wo_iter:
                    f()

            with tc.tile_pool(name="projTab", bufs=1) as tabpool, \
                 tc.tile_pool(name="projX", bufs=10) as xpool, \
                 tc.tile_pool(name="projSW", bufs=3) as swpool, \
                 tc.tile_pool(name="projW", bufs=1) as wpool:
                kvw_es = _ES()
                kvwpool = kvw_es.enter_context(
                    tc.tile_pool(name="projKVW", bufs=1))
                # DMA order matters: the first K-proj matmul needs only
                # wk + the first x quarter, so those go first.
                wk_sb = kvwpool.tile([P, KO, NKV], F32R, tag="wk")
                nc.sync.dma_start(wk_sb[:], wkT_r)
                xqs0 = make_xq(xpool, 0)
                wv_sb = kvwpool.tile([P, KO, NKV], F32R, tag="wv")
                nc.sync.dma_start(wv_sb[:], wvT_r)
                tab_ck = tabpool.tile([P, S], F32, tag="tab_ck")
                tab_sk = tabpool.tile([P, S], F32, tag="tab_sk")
                nc.sync.dma_start(tab_ck[:], ck.ap())
                nc.sync.dma_start(tab_sk[:], sk.ap())
                wq_sb = wpool.tile([P, KO, NQ], F32R, tag="wq")
                nc.sync.dma_start(wq_sb[:], wqT_r)

                # single pass: K, V, Q projections + V' per column chunk
                # (x is read exactly once)
                def kv_proj(n4, xqs):
                    s0 = n4 * 512
                    ps = mmps.tile([P, 512], F32, tag="mm")
                    proj_mm(ps, xqs, wk_sb, 0, NKV, rot=0)
                    rope_evac(ps, kab, s0, tab_ck, tab_sk, swpool, 1.0)
                    ps = mmps.tile([P, 512], F32, tag="mm")
                    proj_mm(ps, xqs, wv_sb, 0, NKV, rot=1)
                    vsb = swpool.tile([P, 512], F32, tag="vsb")
                    nc.scalar.activation(vsb[:], ps[:], COPY)
                    for i in range(2):
                        for kq in range(4):
                            kc = n4 * 4 + kq
                            pst = mmps.tile([P, 512], F32, tag="mm",
                                            name="pst")[:, 0:HD]
                            nc.tensor.transpose(
                                pst[:],
                                vsb[i * HD:(i + 1) * HD,
                                    kq * P:(kq + 1) * P],
                                ident[i * HD:(i + 1) * HD,
                                      i * HD:(i + 1) * HD])
                            nc.scalar.activation(
                                vpr[i][:, kc, 0:HD], pst[:], COPY,
                                scale=msk[:, kc:kc + 1])

                def q_proj4(n4, xqs):
                    s0 = n4 * 512
                    for m in range(4):
                        ps = mmps.tile([P, 512], F32, tag="mm")
                        proj_mm(ps, xqs, wq_sb, m * P, (m + 1) * P,
                                rot=(2 + m) % NKQ)
                        rope_evac(ps, qsb[m], s0, tab_ck, tab_sk,
                                  swpool, 0.125)

                for n4 in range(SC):
                    xqs = xqs0 if n4 == 0 else make_xq(xpool, n4 * 512)
                    kv_proj(n4, xqs)
                    if n4 < 2:
                        q_proj4(n4, xqs)
                for i in range(2):
                    # mask column of V' (ones * mask = mask)
                    nc.vector.tensor_copy(vpr[i][:, :, HD], msk[:])
                kvw_es.close()

                # Q projections for chunks 2,3 become fillers inside
                # attention(0): they run in its ACT-bound PE slack.
                xq_box = {}

                def qfill(n4, m):
                    def f():
                        if m == 0:
                            xq_box[n4] = make_xq(xpool, n4 * 512)
                        ps = mmps.tile([P, 512], F32, tag="mm")
                        proj_mm(ps, xq_box[n4], wq_sb, m * P, (m + 1) * P,
                                rot=(2 + m) % NKQ)
                        rope_evac(ps, qsb[m], n4 * 512, tab_ck, tab_sk,
                                  swpool, 0.125)
                    return f

                attn = qsb
                attention(0, attn,
                          [qfill(n4, m) for n4 in (2, 3) for m in range(4)])

            # q-proj pools closed; wo weights load into the freed space
            with tc.tile_pool(name="wo", bufs=1) as wopool, \
                 tc.tile_pool(name="oev", bufs=4) as oevpool:
                wot_sb = wopool.tile([P, 4, S], F32R, tag="wot_sb")
                nc.sync.dma_start(wot_sb[:], woT_r)
                # wo for qcp0's queries rides inside attention(1)'s
                # ACT-bound phase (PE has slack there)
                attention(1, attn,
                          [(lambda q=qt: wo_chunk(wot_sb, attn, oevpool, q))
                           for qt in range(0, 8)])
                for qt in range(8, KO):
                    wo_chunk(wot_sb, attn, oevpool, qt)

    nc.compile()
    return nc


_PERM = np.concatenate([np.arange(0, HD, 2), np.arange(1, HD, 2)])


def _round_fp32r(a):
    """Round float32 to fp32r (low 12 mantissa bits dropped, nearest-even)."""
    b = np.ascontiguousarray(a, dtype=np.float32).view(np.uint32)
    lsb = (b >> 12) & 1
    out = ((b + 0x7FF + lsb) & 0xFFFFF000).astype(np.uint32)
    return out.view(np.float32)


def _prep_core_inputs(x, wq, wk, wv, wo, attention_mask, core, tables):
    b = core // 4
    g = core % 4
    ctab, stab = tables

    # head order [0,4,1,5,2,6,3,7]: tile m holds heads (m, m+4) so head h
    # sits at partition base (h//4)*64 == its kv head's base in kab
    hperm = np.array([0, 4, 1, 5, 2, 6, 3, 7])
    qrows = wq[8 * g * HD:(8 * g + 8) * HD]          # [512, 2048]
    qrows = qrows.reshape(8, HD, D)[hperm][:, _PERM, :].reshape(NQ, D)
    krows = wk[2 * g * HD:(2 * g + 2) * HD]          # [128, 2048]
    krows = krows.reshape(2, HD, D)[:, _PERM, :].reshape(NKV, D)
    vrows = wv[2 * g * HD:(2 * g + 2) * HD]          # [128, 2048]
    wocols = wo[:, 8 * g * HD:(8 * g + 8) * HD]      # [2048, 512]
    wocols = wocols.reshape(D, 8, HD)[:, hperm, :].reshape(D, NQ)

    maskf = attention_mask[b].astype(np.float32)     # [S]
    maskT = np.ascontiguousarray(maskf.reshape(KO, P).T)   # [128, 16]

    return {
        "xT": _round_fp32r(x[b].T),
        "wqT": _round_fp32r(qrows.T),
        "wkT": _round_fp32r(krows.T),
        "wvT": _round_fp32r(vrows.T),
        "woT": _round_fp32r(wocols.T),
        "ck": ctab,
        "sk": stab,
        "maskT": maskT,
    }


_CACHED_NC = None


def _get_nc():
    global _CACHED_NC
    if _CACHED_NC is None:
        _CACHED_NC = _build_bass()
    return _CACHED_NC


def _make_in_maps(x, wq, wk, wv, wo, attention_mask):
    tables = _rope_tables()
    return [
        _prep_core_inputs(x, wq, wk, wv, wo, attention_mask, c, tables)
        for c in range(N_CORES)
    ]


def kernel(x, wq, wk, wv, wo, attention_mask):
    from concourse.bass_utils import run_bass_kernel_spmd

    x = np.asarray(x, dtype=np.float32)
    wq = np.asarray(wq, dtype=np.float32)
    wk = np.asarray(wk, dtype=np.float32)
    wv = np.asarray(wv, dtype=np.float32)
    wo = np.asarray(wo, dtype=np.float32)
    attention_mask = np.asarray(attention_mask)

    nc = _get_nc()
    in_maps = _make_in_maps(x, wq, wk, wv, wo, attention_mask)
    res = run_bass_kernel_spmd(nc, in_maps, core_ids=list(range(N_CORES)))
    out = np.zeros((B, S, D), dtype=np.float32)
    for c in range(N_CORES):
        out[c // 4] += res.results[c]["part"]
    return out


if __name__ == "__main__":
    rng = np.random.default_rng(0)
    ins = {
        "x": rng.standard_normal((B, S, D), dtype=np.float32),
        "wq": (rng.standard_normal((H * HD, D)) * 0.02).astype(np.float32),
        "wk": (rng.standard_normal((KVH * HD, D)) * 0.02).astype(np.float32),
        "wv": (rng.standard_normal((KVH * HD, D)) * 0.02).astype(np.float32),
        "wo": (rng.standard_normal((D, H * HD)) * 0.02).astype(np.float32),
        "attention_mask": np.ones((B, S), dtype=np.int32),
    }
    out = kernel(**ins)
    print("kernel ran, out shape", out.shape, "std", out.std())


# revision 43
# speedup vs baseline: 1.3489x; 1.3489x over previous
"""GQA attention block (B=2, S=2048, D=2048, H=32, KVH=8, HD=64, RoPE) on 8
Trainium2 NeuronCores.

Sharding: core = (batch, kv-head pair). Core c handles batch c//4 and kv heads
{2*(c%4), 2*(c%4)+1} (i.e. q heads 8*(c%4)..8*(c%4)+7). Each core runs the full
chain for its heads: q/k/v projections + RoPE, attention, and its row-slice of
the output projection; the host sums the 4 partial wo-outputs per batch.

Device-side layout choices (all host-side transforms are free):
- x is passed transposed (xT [D, S]) so projections produce qT/kT/vT with the
  head dim on partitions.
- RoPE uses the "half layout": wq/wk rows are permuted per head to
  [even dims, odd dims] so the rotation pairs live in 32-partition blocks;
  cos/sin tables are precomputed host-side ([128, S] tiles matching the qT/kT
  partition layout). The 1/sqrt(HD) score scale is folded into the Q tables.
- Scores are computed transposed ([keys, queries] on [partitions, free]) so
  softmax exp is a pure elementwise ACT op (no reduction, no transpose of
  probabilities) and the PV matmul consumes probsT directly as the moving
  operand. Softmax skips max-subtraction (scores are bounded ~|7| at this
  problem's scale; fp32 exp is safe).
- The softmax normalizer z is produced by a ones*mask column appended to V
  (M=65 PV matmul); attention output is divided by z after PV (64x less work
  than normalizing probabilities).
- All matmuls run as float32r (full-rate fp32 streaming mode).
"""

import sys

import numpy as np

if "/opt/trn_rl_repo" not in sys.path:
    sys.path.insert(0, "/opt/trn_rl_repo")

B, S, D = 2, 2048, 2048
H, KVH = 32, 8
HD = D // H            # 64
NREP = H // KVH        # 4
ROPE_THETA = 10000.0
N_CORES = 8
P = 128
NQ = 512               # q rows per core (8 heads * 64)
NKV = 128              # k/v rows per core (2 kv heads * 64)
KO = D // P            # 16 contraction chunks for projections
SC = S // 512          # 4 column chunks of 512
HALF = S // 2          # S-half processed per projection pass


def _rope_tables():
    """cos/sin tables [P, S] matching the qT/kT partition layout.

    Partition layout per 64-row head block: rows 0:32 = "a" (even dims),
    rows 32:64 = "b" (odd dims). a' = a*cos - b*sin ; b' = a*sin + b*cos.
    The in0 of the fused swap-multiply reads the OTHER block, so the sin
    table carries -sin on a-rows and +sin on b-rows.

    Computed in float32 to match the reference's jnp float32 math.
    """
    half = HD // 2
    freqs = (1.0 / (ROPE_THETA **
                    (np.arange(0, HD, 2, dtype=np.float32) / np.float32(HD))))
    freqs = freqs.astype(np.float32)                                  # [32]
    ang = (np.arange(S, dtype=np.float32)[None, :] * freqs[:, None])  # [32, S]
    cos = np.cos(ang).astype(np.float32)
    sin = np.sin(ang).astype(np.float32)
    ctab = np.concatenate([cos, cos, cos, cos], axis=0)               # [128, S]
    stab = np.concatenate([-sin, sin, -sin, sin], axis=0)             # [128, S]
    return ctab, stab


def _build_bass(reps: int = 1):
    import concourse.bass as bass  # noqa: F401
    import concourse.tile as tile
    from concourse import bacc, mybir
    from concourse.masks import make_identity

    F32 = mybir.dt.float32
    F32R = mybir.dt.float32r
    EXP = mybir.ActivationFunctionType.Exp
    COPY = mybir.ActivationFunctionType.Copy
    MULT = mybir.AluOpType.mult
    ADD = mybir.AluOpType.add

    nc = bacc.Bacc("TRN2", target_bir_lowering=False, debug=False,
                   num_devices=N_CORES)

    xT = nc.dram_tensor("xT", [D, S], F32R, kind="ExternalInput")
    wqT = nc.dram_tensor("wqT", [D, NQ], F32R, kind="ExternalInput")
    wkT = nc.dram_tensor("wkT", [D, NKV], F32R, kind="ExternalInput")
    wvT = nc.dram_tensor("wvT", [D, NKV], F32R, kind="ExternalInput")
    woT = nc.dram_tensor("woT", [NQ, D], F32R, kind="ExternalInput")
    ck = nc.dram_tensor("ck", [P, S], F32, kind="ExternalInput")
    sk = nc.dram_tensor("sk", [P, S], F32, kind="ExternalInput")
    maskT = nc.dram_tensor("maskT", [P, KO], F32, kind="ExternalInput")
    part = nc.dram_tensor("part", [S, D], F32, kind="ExternalOutput")

    xT_r = xT.ap().rearrange("(ko p) s -> p ko s", p=P)     # [128, 16, 2048]
    wqT_r = wqT.ap().rearrange("(ko p) m -> p ko m", p=P)   # [128, 16, 512]
    wkT_r = wkT.ap().rearrange("(ko p) m -> p ko m", p=P)   # [128, 16, 128]
    wvT_r = wvT.ap().rearrange("(ko p) m -> p ko m", p=P)   # [128, 16, 128]
    woT_r = woT.ap().rearrange("(dk p) e -> p dk e", p=P)   # [128, 4, 2048]

    with tile.TileContext(nc) as tc:
      for rep in range(reps):
        # pools are scoped so per-partition SBUF stays under 192KB:
        # pass1-only data (wk/wv/k-tables/vsb) frees before attention(0);
        # q-proj data (wq/x/q-tables) frees before attention(1); attn/woT
        # allocate late.
        with tc.tile_pool(name="persist", bufs=1) as persist, \
             tc.tile_pool(name="probs", bufs=4) as prpool, \
             tc.tile_pool(name="nrm", bufs=3) as nrmpool, \
             tc.tile_pool(name="mmPS", bufs=2, space="PSUM") as mmps, \
             tc.tile_pool(name="attnPS", bufs=2, space="PSUM") as spool, \
             tc.tile_pool(name="pvPS", bufs=2, space="PSUM") as pvpool:

            from contextlib import ExitStack as _ES
            qsb = [persist.tile([P, S], F32R, tag=f"qsb{m}", name=f"qsb{m}_{rep}")
                   for m in range(4)]
            kab = persist.tile([P, S], F32R, tag="kab")
            vpr = [persist.tile([P, KO, HD + 1], F32R, tag=f"vpr{i}",
                                name=f"vpr{i}_{rep}")
                   for i in range(2)]
            msk = persist.tile([P, KO], F32, tag="msk")
            ident = persist.tile([P, P], F32, tag="ident")

            nc.sync.dma_start(msk[:], maskT.ap())
            make_identity(nc, ident[:])

            def rope_evac(ps, dst_tile, s0, ctab, stab, swpool, scale):
                """dst[:, s0:s0+512] = rope(ps * scale), tables at cols s0.

                The 1/sqrt(HD) query scale folds in as the scalar of
                scalar_tensor_tensor: out = (in0 * scale) * table."""
                dst = dst_tile[:, s0:s0 + 512]
                c_sl = ctab[:, s0:s0 + 512]
                s_sl = stab[:, s0:s0 + 512]
                sw = swpool.tile([P, 512], F32, tag="sw")
                # fused swap+scale+mult: sw[blk] = ps[other]*scale*stab[blk]
                for o in range(0, P, 64):
                    nc.vector.scalar_tensor_tensor(
                        sw[o:o + 32, :], ps[o + 32:o + 64, :], scale,
                        s_sl[o:o + 32, :], MULT, MULT)
                    nc.vector.scalar_tensor_tensor(
                        sw[o + 32:o + 64, :], ps[o:o + 32, :], scale,
                        s_sl[o + 32:o + 64, :], MULT, MULT)
                nc.vector.scalar_tensor_tensor(
                    dst, ps[:], scale, c_sl, MULT, MULT)
                nc.vector.tensor_tensor(dst, dst, sw[:], ADD)

            KH = KO // 8  # x streams in [P, 2, 512] k-eighth tiles
            NKQ = KO // KH

            def make_xq(xpool, s0):
                out = []
                for kq in range(NKQ):
                    xq = xpool.tile([P, KH, 512], F32R, tag="xq",
                                    name=f"xq{kq}")
                    nc.sync.dma_start(
                        xq[:], xT_r[:, kq * KH:(kq + 1) * KH, s0:s0 + 512])
                    out.append(xq)
                return out

            def proj_mm(ps, xqs, w_sb, mlo, mhi, rot=0):
                # rotate the contraction order so the x quarter-tiles free
                # staggered (enables prefetch of the next column chunk)
                ks = [(rot * KH + i) % KO for i in range(KO)]
                for i, k in enumerate(ks):
                    nc.tensor.matmul(
                        ps[:], w_sb[:, k, mlo:mhi],
                        xqs[k // KH][:, k % KH, :],
                        start=(i == 0), stop=(i == KO - 1))

            def wo_chunk(wot_sb, attn, oevpool, qt, on_act=False):
                for n in range(SC):
                    po = mmps.tile([P, 512], F32, tag="mm")
                    for dk in range(4):
                        nc.tensor.matmul(
                            po[:],
                            attn[dk][:, qt * P:(qt + 1) * P],
                            wot_sb[:, dk, n * 512:(n + 1) * 512],
                            start=(dk == 0), stop=(dk == 3))
                    ot = oevpool.tile([P, 512], F32, tag="ot")
                    if on_act:
                        # tail chunks evacuate on ACT (idle after the last exp)
                        nc.scalar.activation(ot[:], po[:], COPY)
                    else:
                        nc.vector.tensor_copy(ot[:], po[:])
                    nc.sync.dma_start(
                        part.ap()[qt * P:(qt + 1) * P,
                                  n * 512:(n + 1) * 512],
                        ot[:])

            def attention(qcp, attn, fillers):
                """Attention for query cols qcp*1024:(qcp+1)*1024; calls one
                filler (wo chunk / deferred q-proj) between head iterations
                so that work rides in the ACT-bound phase's PE slack.

                attn ALIASES qsb: head h's normalized output overwrites the
                very q columns it just consumed (rows qb:qb+64 of tile m);
                later heads touch different rows/columns, so the Tile
                dependency tracker keeps this safe."""
                wo_iter = list(fillers)
                for p in range(4):      # head pair hA=p (base 0), hB=p+4
                  for j2 in range(2):   # the two 512-query chunks of qcp
                    qc = qcp * 2 + j2
                    # pair's score matmuls run concurrently on array row
                    # tiles (0,0)/(64,0): K=64 each, disjoint row groups
                    pvA = pvpool.tile([P, 512], F32, tag="pv",
                                      name=f"pvA_{rep}")
                    pvB = pvpool.tile([P, 512], F32, tag="pv",
                                      name=f"pvB_{rep}")
                    pvs = (pvA, pvB)
                    for kc in range(KO):
                        ss = spool.tile([P, 1024], F32, tag="ss")
                        for i in range(2):
                            nc.tensor.matmul(
                                ss[:, i * 512:(i + 1) * 512],
                                kab[i * HD:(i + 1) * HD,
                                    kc * P:(kc + 1) * P],
                                qsb[p][i * HD:(i + 1) * HD,
                                       qc * 512:(qc + 1) * 512],
                                start=True, stop=True,
                                tile_position=(i * HD, 0))
                        pr = prpool.tile([P, 1024], F32R, tag="pr")
                        nc.scalar.activation(pr[:], ss[:], EXP)
                        for i in range(2):
                            nc.tensor.matmul(
                                pvs[i][0:HD + 1, :],
                                vpr[i][:, kc, :],
                                pr[:, i * 512:(i + 1) * 512],
                                start=(kc == 0), stop=(kc == KO - 1))
                    for i in range(2):
                        qb = i * HD
                        # stage the whole PV result to SBUF in one copy so
                        # the psum bank frees for the next pair immediately
                        # (also: custom-DVE ops misread partition-base != 0
                        # inputs on HW, so the z row must reach base-0 SBUF
                        # before reciprocal).
                        pvs_sb = nrmpool.tile([HD + 1, 512], F32, tag="pvsb")
                        nc.vector.tensor_copy(pvs_sb[:], pvs[i][0:HD + 1, :])
                        zrow = nrmpool.tile([1, 512], F32, tag="zrow")
                        nc.vector.tensor_copy(zrow[:], pvs_sb[HD:HD + 1, :])
                        rz = nrmpool.tile([1, 512], F32, tag="rz")
                        nc.vector.reciprocal_approx_fast(rz[:], zrow[:])
                        rzb = nrmpool.tile([HD, 512], F32, tag="rzb")
                        nc.gpsimd.partition_broadcast(rzb[:], rz[:])
                        nc.vector.tensor_tensor(
                            attn[p][qb:qb + HD, qc * 512:(qc + 1) * 512],
                            pvs_sb[0:HD, :], rzb[:], MULT)
                    if wo_iter:
                        wo_iter.pop(0)()
                for f in wo_iter:
                    f()

            with tc.tile_pool(name="projTab", bufs=1) as tabpool, \
                 tc.tile_pool(name="projX", bufs=10) as xpool, \
                 tc.tile_pool(name="projSW", bufs=3) as swpool, \
                 tc.tile_pool(name="projW", bufs=1) as wpool:
                kvw_es = _ES()
                kvwpool = kvw_es.enter_context(
                    tc.tile_pool(name="projKVW", bufs=1))
                # DMA order matters: the first K-proj matmul needs only
                # wk + the first x quarter, so those go first.
                wk_sb = kvwpool.tile([P, KO, NKV], F32R, tag="wk")
                nc.sync.dma_start(wk_sb[:], wkT_r)
                xqs0 = make_xq(xpool, 0)
                wv_sb = kvwpool.tile([P, KO, NKV], F32R, tag="wv")
                nc.sync.dma_start(wv_sb[:], wvT_r)
                tab_ck = tabpool.tile([P, S], F32, tag="tab_ck")
                tab_sk = tabpool.tile([P, S], F32, tag="tab_sk")
                nc.sync.dma_start(tab_ck[:], ck.ap())
                nc.sync.dma_start(tab_sk[:], sk.ap())
                wq_sb = wpool.tile([P, KO, NQ], F32R, tag="wq")
                nc.sync.dma_start(wq_sb[:], wqT_r)

                # single pass: K, V, Q projections + V' per column chunk
                # (x is read exactly once)
                def kv_proj(n4, xqs):
                    s0 = n4 * 512
                    ps = mmps.tile([P, 512], F32, tag="mm")
                    proj_mm(ps, xqs, wk_sb, 0, NKV, rot=0)
                    rope_evac(ps, kab, s0, tab_ck, tab_sk, swpool, 1.0)
                    ps = mmps.tile([P, 512], F32, tag="mm")
                    proj_mm(ps, xqs, wv_sb, 0, NKV, rot=1)
                    vsb = swpool.tile([P, 512], F32, tag="vsb")
                    nc.scalar.activation(vsb[:], ps[:], COPY)
                    for i in range(2):
                        for kq in range(4):
                            kc = n4 * 4 + kq
                            pst = mmps.tile([P, 512], F32, tag="mm",
                                            name="pst")[:, 0:HD]
                            nc.tensor.transpose(
                                pst[:],
                                vsb[i * HD:(i + 1) * HD,
                                    kq * P:(kq + 1) * P],
                                ident[i * HD:(i + 1) * HD,
                                      i * HD:(i + 1) * HD])
                            nc.scalar.activation(
                                vpr[i][:, kc, 0:HD], pst[:], COPY,
                                scale=msk[:, kc:kc + 1])

                def q_proj4(n4, xqs):
                    s0 = n4 * 512
                    for m in range(4):
                        ps = mmps.tile([P, 512], F32, tag="mm")
                        proj_mm(ps, xqs, wq_sb, m * P, (m + 1) * P,
                                rot=(2 + m) % NKQ)
                        rope_evac(ps, qsb[m], s0, tab_ck, tab_sk,
                                  swpool, 0.125)

                for n4 in range(SC):
                    xqs = xqs0 if n4 == 0 else make_xq(xpool, n4 * 512)
                    kv_proj(n4, xqs)
                    if n4 < 2:
                        q_proj4(n4, xqs)
                for i in range(2):
                    # mask column of V' (ones * mask = mask)
                    nc.vector.tensor_copy(vpr[i][:, :, HD], msk[:])
                kvw_es.close()

                # Q projections for chunks 2,3 become fillers inside
                # attention(0): they run in its ACT-bound PE slack.
                xq_box = {}

                def qfill(n4, m):
                    def f():
                        if m == 0:
                            xq_box[n4] = make_xq(xpool, n4 * 512)
                        ps = mmps.tile([P, 512], F32, tag="mm")
                        proj_mm(ps, xq_box[n4], wq_sb, m * P, (m + 1) * P,
                                rot=(2 + m) % NKQ)
                        rope_evac(ps, qsb[m], n4 * 512, tab_ck, tab_sk,
                                  swpool, 0.125)
                    return f

                attn = qsb
                attention(0, attn,
                          [qfill(n4, m) for n4 in (2, 3) for m in range(4)])

            # q-proj pools closed; wo weights load into the freed space
            with tc.tile_pool(name="wo", bufs=1) as wopool, \
                 tc.tile_pool(name="oev", bufs=4) as oevpool:
                wot_sb = wopool.tile([P, 4, S], F32R, tag="wot_sb")
                nc.sync.dma_start(wot_sb[:], woT_r)
                # wo for qcp0's queries rides inside attention(1)'s
                # ACT-bound phase (PE has slack there)
                attention(1, attn,
                          [(lambda q=qt: wo_chunk(wot_sb, attn, oevpool, q))
                           for qt in range(0, 8)])
                for qt in range(8, KO):
                    wo_chunk(wot_sb, attn, oevpool, qt, on_act=True)

    nc.compile()
    return nc


_PERM = np.concatenate([np.arange(0, HD, 2), np.arange(1, HD, 2)])


def _round_fp32r(a):
    """Round float32 to fp32r (low 12 mantissa bits dropped, nearest-even)."""
    b = np.ascontiguousarray(a, dtype=np.float32).view(np.uint32)
    lsb = (b >> 12) & 1
    out = ((b + 0x7FF + lsb) & 0xFFFFF000).astype(np.uint32)
    return out.view(np.float32)


def _prep_core_inputs(x, wq, wk, wv, wo, attention_mask, core, tables):
    b = core // 4
    g = core % 4
    ctab, stab = tables

    # head order [0,4,1,5,2,6,3,7]: tile m holds heads (m, m+4) so head h
    # sits at partition base (h//4)*64 == its kv head's base in kab
    hperm = np.array([0, 4, 1, 5, 2, 6, 3, 7])
    qrows = wq[8 * g * HD:(8 * g + 8) * HD]          # [512, 2048]
    qrows = qrows.reshape(8, HD, D)[hperm][:, _PERM, :].reshape(NQ, D)
    krows = wk[2 * g * HD:(2 * g + 2) * HD]          # [128, 2048]
    krows = krows.reshape(2, HD, D)[:, _PERM, :].reshape(NKV, D)
    vrows = wv[2 * g * HD:(2 * g + 2) * HD]          # [128, 2048]
    wocols = wo[:, 8 * g * HD:(8 * g + 8) * HD]      # [2048, 512]
    wocols = wocols.reshape(D, 8, HD)[:, hperm, :].reshape(D, NQ)

    maskf = attention_mask[b].astype(np.float32)     # [S]
    maskT = np.ascontiguousarray(maskf.reshape(KO, P).T)   # [128, 16]

    return {
        "xT": _round_fp32r(x[b].T),
        "wqT": _round_fp32r(qrows.T),
        "wkT": _round_fp32r(krows.T),
        "wvT": _round_fp32r(vrows.T),
        "woT": _round_fp32r(wocols.T),
        "ck": ctab,
        "sk": stab,
        "maskT": maskT,
    }


_CACHED_NC = None


def _get_nc():
    global _CACHED_NC
    if _CACHED_NC is None:
        _CACHED_NC = _build_bass()
    return _CACHED_NC


def _make_in_maps(x, wq, wk, wv, wo, attention_mask):
    tables = _rope_tables()
    return [
        _prep_core_inputs(x, wq, wk, wv, wo, attention_mask, c, tables)
        for c in range(N_CORES)
    ]


def kernel(x, wq, wk, wv, wo, attention_mask):
    from concourse.bass_utils import run_bass_kernel_spmd

    x = np.asarray(x, dtype=np.float32)
    wq = np.asarray(wq, dtype=np.float32)
    wk = np.asarray(wk, dtype=np.float32)
    wv = np.asarray(wv, dtype=np.float32)
    wo = np.asarray(wo, dtype=np.float32)
    attention_mask = np.asarray(attention_mask)

    nc = _get_nc()
    in_maps = _make_in_maps(x, wq, wk, wv, wo, attention_mask)
    res = run_bass_kernel_spmd(nc, in_maps, core_ids=list(range(N_CORES)))
    out = np.zeros((B, S, D), dtype=np.float32)
    for c in range(N_CORES):
        out[c // 4] += res.results[c]["part"]
    return out


if __name__ == "__main__":
    rng = np.random.default_rng(0)
    ins = {
        "x": rng.standard_normal((B, S, D), dtype=np.float32),
        "wq": (rng.standard_normal((H * HD, D)) * 0.02).astype(np.float32),
        "wk": (rng.standard_normal((KVH * HD, D)) * 0.02).astype(np.float32),
        "wv": (rng.standard_normal((KVH * HD, D)) * 0.02).astype(np.float32),
        "wo": (rng.standard_normal((D, H * HD)) * 0.02).astype(np.float32),
        "attention_mask": np.ones((B, S), dtype=np.int32),
    }
    out = kernel(**ins)
    print("kernel ran, out shape", out.shape, "std", out.std())


# revision 49
# speedup vs baseline: 1.3822x; 1.0247x over previous
"""GQA attention block (B=2, S=2048, D=2048, H=32, KVH=8, HD=64, RoPE) on 8
Trainium2 NeuronCores.

Sharding: core = (batch, kv-head pair). Core c handles batch c//4 and kv heads
{2*(c%4), 2*(c%4)+1} (i.e. q heads 8*(c%4)..8*(c%4)+7). Each core runs the full
chain for its heads: q/k/v projections + RoPE, attention, and its row-slice of
the output projection; the host sums the 4 partial wo-outputs per batch.

Device-side layout choices (all host-side transforms are free):
- x is passed transposed (xT [D, S]) so projections produce qT/kT/vT with the
  head dim on partitions.
- RoPE uses the "half layout": wq/wk rows are permuted per head to
  [even dims, odd dims] so the rotation pairs live in 32-partition blocks;
  cos/sin tables are precomputed host-side ([128, S] tiles matching the qT/kT
  partition layout). The 1/sqrt(HD) score scale is folded into the Q tables.
- Scores are computed transposed ([keys, queries] on [partitions, free]) so
  softmax exp is a pure elementwise ACT op (no reduction, no transpose of
  probabilities) and the PV matmul consumes probsT directly as the moving
  operand. Softmax skips max-subtraction (scores are bounded ~|7| at this
  problem's scale; fp32 exp is safe).
- The softmax normalizer z is produced by a ones*mask column appended to V
  (M=65 PV matmul); attention output is divided by z after PV (64x less work
  than normalizing probabilities).
- All matmuls run as float32r (full-rate fp32 streaming mode).
"""

import sys

import numpy as np

if "/opt/trn_rl_repo" not in sys.path:
    sys.path.insert(0, "/opt/trn_rl_repo")

B, S, D = 2, 2048, 2048
H, KVH = 32, 8
HD = D // H            # 64
NREP = H // KVH        # 4
ROPE_THETA = 10000.0
N_CORES = 8
P = 128
NQ = 512               # q rows per core (8 heads * 64)
NKV = 128              # k/v rows per core (2 kv heads * 64)
KO = D // P            # 16 contraction chunks for projections
SC = S // 512          # 4 column chunks of 512
HALF = S // 2          # S-half processed per projection pass


def _rope_tables():
    """cos/sin tables [P, S] matching the qT/kT partition layout.

    Partition layout per 64-row head block: rows 0:32 = "a" (even dims),
    rows 32:64 = "b" (odd dims). a' = a*cos - b*sin ; b' = a*sin + b*cos.
    The in0 of the fused swap-multiply reads the OTHER block, so the sin
    table carries -sin on a-rows and +sin on b-rows.

    Computed in float32 to match the reference's jnp float32 math.
    """
    half = HD // 2
    freqs = (1.0 / (ROPE_THETA **
                    (np.arange(0, HD, 2, dtype=np.float32) / np.float32(HD))))
    freqs = freqs.astype(np.float32)                                  # [32]
    ang = (np.arange(S, dtype=np.float32)[None, :] * freqs[:, None])  # [32, S]
    cos = np.cos(ang).astype(np.float32)
    sin = np.sin(ang).astype(np.float32)
    ctab = np.concatenate([cos, cos, cos, cos], axis=0)               # [128, S]
    stab = np.concatenate([-sin, sin, -sin, sin], axis=0)             # [128, S]
    return ctab, stab


def _build_bass(reps: int = 1):
    import concourse.bass as bass  # noqa: F401
    import concourse.tile as tile
    from concourse import bacc, mybir
    from concourse.masks import make_identity

    F32 = mybir.dt.float32
    F32R = mybir.dt.float32r
    EXP = mybir.ActivationFunctionType.Exp
    COPY = mybir.ActivationFunctionType.Copy
    MULT = mybir.AluOpType.mult
    ADD = mybir.AluOpType.add

    nc = bacc.Bacc("TRN2", target_bir_lowering=False, debug=False,
                   num_devices=N_CORES)

    xT = nc.dram_tensor("xT", [D, S], F32R, kind="ExternalInput")
    wqT = nc.dram_tensor("wqT", [D, NQ], F32R, kind="ExternalInput")
    wkT = nc.dram_tensor("wkT", [D, NKV], F32R, kind="ExternalInput")
    wvT = nc.dram_tensor("wvT", [D, NKV], F32R, kind="ExternalInput")
    woT = nc.dram_tensor("woT", [NQ, D], F32R, kind="ExternalInput")
    ck = nc.dram_tensor("ck", [P, S], F32, kind="ExternalInput")
    sk = nc.dram_tensor("sk", [P, S], F32, kind="ExternalInput")
    maskT = nc.dram_tensor("maskT", [P, KO], F32, kind="ExternalInput")
    part = nc.dram_tensor("part", [S, D], F32, kind="ExternalOutput")

    xT_r = xT.ap().rearrange("(ko p) s -> p ko s", p=P)     # [128, 16, 2048]
    wqT_r = wqT.ap().rearrange("(ko p) m -> p ko m", p=P)   # [128, 16, 512]
    wkT_r = wkT.ap().rearrange("(ko p) m -> p ko m", p=P)   # [128, 16, 128]
    wvT_r = wvT.ap().rearrange("(ko p) m -> p ko m", p=P)   # [128, 16, 128]
    woT_r = woT.ap().rearrange("(dk p) e -> p dk e", p=P)   # [128, 4, 2048]

    with tile.TileContext(nc) as tc:
      for rep in range(reps):
        # pools are scoped so per-partition SBUF stays under 192KB:
        # pass1-only data (wk/wv/k-tables/vsb) frees before attention(0);
        # q-proj data (wq/x/q-tables) frees before attention(1); attn/woT
        # allocate late.
        with tc.tile_pool(name="persist", bufs=1) as persist, \
             tc.tile_pool(name="probs", bufs=4) as prpool, \
             tc.tile_pool(name="nrm", bufs=3) as nrmpool, \
             tc.tile_pool(name="mmPS", bufs=2, space="PSUM") as mmps, \
             tc.tile_pool(name="attnPS", bufs=2, space="PSUM") as spool, \
             tc.tile_pool(name="pvPS", bufs=2, space="PSUM") as pvpool:

            from contextlib import ExitStack as _ES
            qsb = [persist.tile([P, S], F32R, tag=f"qsb{m}", name=f"qsb{m}_{rep}")
                   for m in range(4)]
            kab = persist.tile([P, S], F32R, tag="kab")
            vpr = [persist.tile([P, KO, HD + 1], F32R, tag=f"vpr{i}",
                                name=f"vpr{i}_{rep}")
                   for i in range(2)]
            msk = persist.tile([P, KO], F32, tag="msk")
            ident = persist.tile([P, P], F32, tag="ident")

            nc.sync.dma_start(msk[:], maskT.ap())
            make_identity(nc, ident[:])

            def rope_evac(ps, dst_tile, s0, ctab, stab, swpool, scale):
                """dst[:, s0:s0+512] = rope(ps * scale), tables at cols s0.

                The 1/sqrt(HD) query scale folds in as the scalar of
                scalar_tensor_tensor: out = (in0 * scale) * table."""
                dst = dst_tile[:, s0:s0 + 512]
                c_sl = ctab[:, s0:s0 + 512]
                s_sl = stab[:, s0:s0 + 512]
                sw = swpool.tile([P, 512], F32, tag="sw")
                # fused swap+scale+mult: sw[blk] = ps[other]*scale*stab[blk]
                for o in range(0, P, 64):
                    nc.vector.scalar_tensor_tensor(
                        sw[o:o + 32, :], ps[o + 32:o + 64, :], scale,
                        s_sl[o:o + 32, :], MULT, MULT)
                    nc.vector.scalar_tensor_tensor(
                        sw[o + 32:o + 64, :], ps[o:o + 32, :], scale,
                        s_sl[o + 32:o + 64, :], MULT, MULT)
                nc.vector.scalar_tensor_tensor(
                    dst, ps[:], scale, c_sl, MULT, MULT)
                nc.vector.tensor_tensor(dst, dst, sw[:], ADD)

            KH = KO // 8  # x streams in [P, 2, 512] k-eighth tiles
            NKQ = KO // KH

            def make_xq(xpool, s0):
                out = []
                for kq in range(NKQ):
                    xq = xpool.tile([P, KH, 512], F32R, tag="xq",
                                    name=f"xq{kq}")
                    nc.sync.dma_start(
                        xq[:], xT_r[:, kq * KH:(kq + 1) * KH, s0:s0 + 512])
                    out.append(xq)
                return out

            def proj_mm(ps, xqs, w_sb, mlo, mhi, rot=0):
                # rotate the contraction order so the x quarter-tiles free
                # staggered (enables prefetch of the next column chunk)
                ks = [(rot * KH + i) % KO for i in range(KO)]
                for i, k in enumerate(ks):
                    nc.tensor.matmul(
                        ps[:], w_sb[:, k, mlo:mhi],
                        xqs[k // KH][:, k % KH, :],
                        start=(i == 0), stop=(i == KO - 1))

            def wo_chunk(wot_sb, attn, oevpool, qt, on_act=False):
                for n in range(SC):
                    po = mmps.tile([P, 512], F32, tag="mm")
                    for dk in range(4):
                        nc.tensor.matmul(
                            po[:],
                            attn[dk][:, qt * P:(qt + 1) * P],
                            wot_sb[:, dk, n * 512:(n + 1) * 512],
                            start=(dk == 0), stop=(dk == 3))
                    ot = oevpool.tile([P, 512], F32, tag="ot")
                    if on_act:
                        # tail chunks evacuate on ACT (idle after the last exp)
                        nc.scalar.activation(ot[:], po[:], COPY)
                    else:
                        nc.vector.tensor_copy(ot[:], po[:])
                    nc.sync.dma_start(
                        part.ap()[qt * P:(qt + 1) * P,
                                  n * 512:(n + 1) * 512],
                        ot[:])

            def attention(qcp, attn, fillers):
                """Attention for query cols qcp*1024:(qcp+1)*1024; calls one
                filler (wo chunk / deferred q-proj) between head iterations
                so that work rides in the ACT-bound phase's PE slack.

                attn ALIASES qsb: head h's normalized output overwrites the
                very q columns it just consumed (rows qb:qb+64 of tile m);
                later heads touch different rows/columns, so the Tile
                dependency tracker keeps this safe."""
                wo_iter = list(fillers)
                for p in range(4):      # head pair hA=p (base 0), hB=p+4
                  for j2 in range(2):   # the two 512-query chunks of qcp
                    qc = qcp * 2 + j2
                    # pair's score matmuls run concurrently on array row
                    # tiles (0,0)/(64,0): K=64 each, disjoint row groups
                    pvA = pvpool.tile([P, 512], F32, tag="pv",
                                      name=f"pvA_{rep}")
                    pvB = pvpool.tile([P, 512], F32, tag="pv",
                                      name=f"pvB_{rep}")
                    pvs = (pvA, pvB)
                    for kc in range(KO):
                        ss = spool.tile([P, 1024], F32, tag="ss")
                        for i in range(2):
                            nc.tensor.matmul(
                                ss[:, i * 512:(i + 1) * 512],
                                kab[i * HD:(i + 1) * HD,
                                    kc * P:(kc + 1) * P],
                                qsb[p][i * HD:(i + 1) * HD,
                                       qc * 512:(qc + 1) * 512],
                                start=True, stop=True,
                                tile_position=(i * HD, 0))
                        pr = prpool.tile([P, 1024], F32R, tag="pr")
                        nc.scalar.activation(pr[:], ss[:], EXP)
                        for i in range(2):
                            nc.tensor.matmul(
                                pvs[i][0:HD + 1, :],
                                vpr[i][:, kc, :],
                                pr[:, i * 512:(i + 1) * 512],
                                start=(kc == 0), stop=(kc == KO - 1))
                    for i in range(2):
                        qb = i * HD
                        # stage the whole PV result to SBUF in one copy so
                        # the psum bank frees for the next pair immediately
                        # (also: custom-DVE ops misread partition-base != 0
                        # inputs on HW, so the z row must reach base-0 SBUF
                        # before reciprocal).
                        pvs_sb = nrmpool.tile([HD + 1, 512], F32, tag="pvsb")
                        nc.vector.tensor_copy(pvs_sb[:], pvs[i][0:HD + 1, :])
                        zrow = nrmpool.tile([1, 512], F32, tag="zrow")
                        nc.vector.tensor_copy(zrow[:], pvs_sb[HD:HD + 1, :])
                        rz = nrmpool.tile([1, 512], F32, tag="rz")
                        nc.vector.reciprocal_approx_fast(rz[:], zrow[:])
                        rzb = nrmpool.tile([HD, 512], F32, tag="rzb")
                        nc.gpsimd.partition_broadcast(rzb[:], rz[:])
                        nc.vector.tensor_tensor(
                            attn[p][qb:qb + HD, qc * 512:(qc + 1) * 512],
                            pvs_sb[0:HD, :], rzb[:], MULT)
                    if wo_iter:
                        wo_iter.pop(0)()
                for f in wo_iter:
                    f()

            with tc.tile_pool(name="projTab", bufs=1) as tabpool, \
                 tc.tile_pool(name="projX", bufs=10) as xpool, \
                 tc.tile_pool(name="projSW", bufs=3) as swpool, \
                 tc.tile_pool(name="projW", bufs=1) as wpool:
                kvw_es = _ES()
                kvwpool = kvw_es.enter_context(
                    tc.tile_pool(name="projKVW", bufs=1))
                # DMA order matters: the first K-proj matmul needs only
                # wk + the first x quarter, so those go first.
                wk_sb = kvwpool.tile([P, KO, NKV], F32R, tag="wk")
                nc.sync.dma_start(wk_sb[:], wkT_r)
                xqs0 = make_xq(xpool, 0)
                wv_sb = kvwpool.tile([P, KO, NKV], F32R, tag="wv")
                nc.sync.dma_start(wv_sb[:], wvT_r)
                tab_ck = tabpool.tile([P, S], F32, tag="tab_ck")
                tab_sk = tabpool.tile([P, S], F32, tag="tab_sk")
                nc.sync.dma_start(tab_ck[:], ck.ap())
                nc.sync.dma_start(tab_sk[:], sk.ap())
                wq_sb = wpool.tile([P, KO, NQ], F32R, tag="wq")
                nc.sync.dma_start(wq_sb[:], wqT_r)

                # single pass: K, V, Q projections + V' per column chunk
                # (x is read exactly once)
                def kv_proj(n4, xqs):
                    s0 = n4 * 512
                    ps = mmps.tile([P, 512], F32, tag="mm")
                    proj_mm(ps, xqs, wk_sb, 0, NKV, rot=0)
                    rope_evac(ps, kab, s0, tab_ck, tab_sk, swpool, 1.0)
                    ps = mmps.tile([P, 512], F32, tag="mm")
                    proj_mm(ps, xqs, wv_sb, 0, NKV, rot=1)
                    vsb = swpool.tile([P, 512], F32, tag="vsb")
                    nc.scalar.activation(vsb[:], ps[:], COPY)
                    for i in range(2):
                        for kq in range(4):
                            kc = n4 * 4 + kq
                            pst = mmps.tile([P, 512], F32, tag="mm",
                                            name="pst")[:, 0:HD]
                            nc.tensor.transpose(
                                pst[:],
                                vsb[i * HD:(i + 1) * HD,
                                    kq * P:(kq + 1) * P],
                                ident[i * HD:(i + 1) * HD,
                                      i * HD:(i + 1) * HD])
                            nc.scalar.activation(
                                vpr[i][:, kc, 0:HD], pst[:], COPY,
                                scale=msk[:, kc:kc + 1])

                def q_proj4(n4, xqs):
                    s0 = n4 * 512
                    for m in range(4):
                        ps = mmps.tile([P, 512], F32, tag="mm")
                        proj_mm(ps, xqs, wq_sb, m * P, (m + 1) * P,
                                rot=(2 + m) % NKQ)
                        rope_evac(ps, qsb[m], s0, tab_ck, tab_sk,
                                  swpool, 0.125)

                for n4 in range(SC):
                    xqs = xqs0 if n4 == 0 else make_xq(xpool, n4 * 512)
                    kv_proj(n4, xqs)
                    if n4 < 2:
                        q_proj4(n4, xqs)
                for i in range(2):
                    # mask column of V' (ones * mask = mask)
                    nc.vector.tensor_copy(vpr[i][:, :, HD], msk[:])
                kvw_es.close()

                # Q projections for chunks 2,3 become fillers inside
                # attention(0): they run in its ACT-bound PE slack.
                xq_box = {}

                def qfill(n4, m):
                    def f():
                        if m == 0:
                            xq_box[n4] = make_xq(xpool, n4 * 512)
                        ps = mmps.tile([P, 512], F32, tag="mm")
                        proj_mm(ps, xq_box[n4], wq_sb, m * P, (m + 1) * P,
                                rot=(2 + m) % NKQ)
                        rope_evac(ps, qsb[m], n4 * 512, tab_ck, tab_sk,
                                  swpool, 0.125)
                    return f

                attn = qsb
                attention(0, attn,
                          [qfill(n4, m) for n4 in (2, 3) for m in range(4)])

            # q-proj pools closed; wo weights load into the freed space
            with tc.tile_pool(name="wo", bufs=1) as wopool, \
                 tc.tile_pool(name="oev", bufs=4) as oevpool:
                wot_sb = wopool.tile([P, 4, S], F32R, tag="wot_sb")
                nc.sync.dma_start(wot_sb[:], woT_r)
                # wo for qcp0's queries rides inside attention(1)'s
                # ACT-bound phase (PE has slack there)
                attention(1, attn,
                          [(lambda q=qt: wo_chunk(wot_sb, attn, oevpool, q))
                           for qt in range(0, 8)])
                for qt in range(8, KO):
                    wo_chunk(wot_sb, attn, oevpool, qt, on_act=True)

    nc.compile()
    return nc


_PERM = np.concatenate([np.arange(0, HD, 2), np.arange(1, HD, 2)])


def _round_fp32r(a):
    """Round float32 to fp32r (low 12 mantissa bits dropped, nearest-even)."""
    b = np.ascontiguousarray(a, dtype=np.float32).view(np.uint32)
    lsb = (b >> 12) & 1
    out = ((b + 0x7FF + lsb) & 0xFFFFF000).astype(np.uint32)
    return out.view(np.float32)


def _prep_core_inputs(x, wq, wk, wv, wo, attention_mask, core, tables):
    b = core // 4
    g = core % 4
    ctab, stab = tables

    # head order [0,4,1,5,2,6,3,7]: tile m holds heads (m, m+4) so head h
    # sits at partition base (h//4)*64 == its kv head's base in kab
    hperm = np.array([0, 4, 1, 5, 2, 6, 3, 7])
    qrows = wq[8 * g * HD:(8 * g + 8) * HD]          # [512, 2048]
    qrows = qrows.reshape(8, HD, D)[hperm][:, _PERM, :].reshape(NQ, D)
    krows = wk[2 * g * HD:(2 * g + 2) * HD]          # [128, 2048]
    krows = krows.reshape(2, HD, D)[:, _PERM, :].reshape(NKV, D)
    vrows = wv[2 * g * HD:(2 * g + 2) * HD]          # [128, 2048]
    wocols = wo[:, 8 * g * HD:(8 * g + 8) * HD]      # [2048, 512]
    wocols = wocols.reshape(D, 8, HD)[:, hperm, :].reshape(D, NQ)

    maskf = attention_mask[b].astype(np.float32)     # [S]
    maskT = np.ascontiguousarray(maskf.reshape(KO, P).T)   # [128, 16]

    return {
        "xT": _round_fp32r(x[b].T),
        "wqT": _round_fp32r(qrows.T),
        "wkT": _round_fp32r(krows.T),
        "wvT": _round_fp32r(vrows.T),
        "woT": _round_fp32r(wocols.T),
        "ck": ctab,
        "sk": stab,
        "maskT": maskT,
    }


_CACHED_NC = None


def _get_nc():
    global _CACHED_NC
    if _CACHED_NC is None:
        _CACHED_NC = _build_bass()
    return _CACHED_NC


def _make_in_maps(x, wq, wk, wv, wo, attention_mask):
    tables = _rope_tables()
    return [
        _prep_core_inputs(x, wq, wk, wv, wo, attention_mask, c, tables)
        for c in range(N_CORES)
    ]


def kernel(x, wq, wk, wv, wo, attention_mask):
    from concourse.bass_utils import run_bass_kernel_spmd

    x = np.asarray(x, dtype=np.float32)
    wq = np.asarray(wq, dtype=np.float32)
    wk = np.asarray(wk, dtype=np.float32)
    wv = np.asarray(wv, dtype=np.float32)
    wo = np.asarray(wo, dtype=np.float32)
    attention_mask = np.asarray(attention_mask)

    nc = _get_nc()
    in_maps = _make_in_maps(x, wq, wk, wv, wo, attention_mask)
    res = run_bass_kernel_spmd(nc, in_maps, core_ids=list(range(N_CORES)))
    out = np.zeros((B, S, D), dtype=np.float32)
    for c in range(N_CORES):
        out[c // 4] += res.results[c]["part"]
    return out


if __name__ == "__main__":
    rng = np.random.default_rng(0)
    ins = {
        "x": rng.standard_normal((B, S, D), dtype=np.float32),
        "wq": (rng.standard_normal((H * HD, D)) * 0.02).astype(np.float32),
        "wk": (rng.standard_normal((KVH * HD, D)) * 0.02).astype(np.float32),
        "wv": (rng.standard_normal((KVH * HD, D)) * 0.02).astype(np.float32),
        "wo": (rng.standard_normal((D, H * HD)) * 0.02).astype(np.float32),
        "attention_mask": np.ones((B, S), dtype=np.int32),
    }
    out = kernel(**ins)
    print("kernel ran, out shape", out.shape, "std", out.std())
